# revision 16
# baseline (speedup 1.0000x reference)
"""Patch-correlation argmax (retrieval KNN) on 8 Trainium2 NeuronCores.

Pipeline:
  host:   3x3 unfold of both images -> [B, 576, 9216] patch matrices,
          l2-normalize the ref patches, cast to bf16, pad K 576->640.
  device: shard the lr-patch axis (n) across 8 cores. Each core computes
          R tiles [128 n x 512 m] as 5 PSUM-accumulated bf16 matmuls
          (K=640 in 5 chunks of 128) and reduces each tile to its top-8
          values + indices with the DVE Max8/MaxIndex ops. No R tile ever
          touches HBM.
  host:   merge the per-tile top-8 candidate lists (144 per output pixel),
          rescore the best 16 in float64 against the exact fp32 patch
          data, and emit S (max corr) + Hmap (argmax index).
"""

import numpy as np
import ml_dtypes

B, C, H, W = 2, 64, 96, 96
HW = H * W            # 9216 patches per image
K = C * 9             # 576 contraction dim
KC = 5                # K chunks of 128 (576 zero-padded to 640)
NCORES = 8
NPC = HW // NCORES    # 1152 lr patches per core
NT = NPC // 128       # 9 n-tiles of 128 per core
MTILE = 512
MT = HW // MTILE      # 18 m-tiles
NCAND = MT * 8        # 144 candidates per pixel
RESCORE = 16          # candidates rescored exactly on host
EPS = 1e-12


def _unfold(x):
    """[B,C,H,W] f32 -> [B, C*9, H*W] matching F.unfold(kernel=3, padding=1)."""
    xp = np.zeros((B, C, H + 2, W + 2), np.float32)
    xp[:, :, 1:-1, 1:-1] = x
    out = np.empty((B, C, 9, HW), np.float32)
    for di in range(3):
        for dj in range(3):
            out[:, :, di * 3 + dj] = xp[:, :, di:di + H, dj:dj + W].reshape(B, C, HW)
    return out.reshape(B, K, HW)


def _to_chunks_bf16(u):
    """[B, 576, HW'] f32 -> [B, 128, 5, HW'] bf16, K zero-padded to 640.

    Device layout: partition p of chunk kc holds original k = kc*128 + p.
    """
    n = u.shape[2]
    up = np.zeros((B, KC * 128, n), np.float32)
    up[:, :K] = u
    return np.ascontiguousarray(
        up.reshape(B, KC, 128, n).transpose(0, 2, 1, 3)
    ).astype(ml_dtypes.bfloat16)


def _cap_sync_waits(nc):
    """Walrus instruction structs hold a single sync-wait slot (matmul, DMA).

    Tile sometimes emits 2 waits on one instruction (input RAW + slot WAR).
    Split the excess into standalone EventSemaphore waits inserted directly
    before the instruction on the same engine queue — semantically identical
    (every wait still holds at the same queue position), always compilable.
    """
    import concourse.mybir as mybir

    n = 0
    for blk in nc.m.functions[0].blocks:
        ins_list = list(blk.instructions)
        out, changed = [], False
        for ins in ins_list:
            si = ins.sync_info
            if (si is not None and len(si.on_wait) > 1
                    and type(ins).__name__ != "InstEventSemaphore"):
                for w in si.on_wait[:-1]:
                    n += 1
                    out.append(mybir.InstEventSemaphore(
                        name=f"bridge_wait_{n}",
                        engine=ins.engine,
                        sync_info=mybir.SyncInfo(on_wait=[w], on_update=[]),
                    ))
                si.on_wait[:] = si.on_wait[-1:]
                changed = True
            out.append(ins)
        if changed:
            blk.instructions = out


def _build_bass(fix_waits=True):
    import concourse.bass as bass
    import concourse.mybir as mybir
    from concourse.tile import TileContext

    nc = bass.Bass()
    refk = nc.declare_dram_parameter(
        "refk", [B, 128, KC, HW], mybir.dt.bfloat16, isOutput=False)
    lrk = nc.declare_dram_parameter(
        "lrk", [B, 128, KC, NPC], mybir.dt.bfloat16, isOutput=False)
    vals = nc.declare_dram_parameter(
        "vals", [B, 128, NT, NCAND], mybir.dt.float32, isOutput=True)
    idxs = nc.declare_dram_parameter(
        "idxs", [B, 128, NT, NCAND], mybir.dt.uint16, isOutput=True)

    with TileContext(nc) as tc:
        with (
            tc.tile_pool(name="big", bufs=1) as big,
            tc.tile_pool(name="io", bufs=2) as io,
            tc.tile_pool(name="ps", bufs=4, space="PSUM") as psp,
            tc.tile_pool(name="ps1", bufs=1, space="PSUM") as psp1,
        ):
            for b in range(B):
                # Tile's dependency tracking is per-tile, so ref gets one
                # tile per m-slice (single DMA writer each) rather than 18
                # partial writes into one big tile: matmuls on slice mt can
                # start as soon as that slice has landed.
                lr_t = big.tile([128, KC, NPC], mybir.dt.bfloat16, tag="lr")
                nc.sync.dma_start(lr_t[:], lrk[b])
                ref_ts = []
                for mt in range(MT):
                    rt = big.tile([128, KC, MTILE], mybir.dt.bfloat16,
                                  tag=f"ref{mt}")
                    sl = slice(mt * MTILE, (mt + 1) * MTILE)
                    nc.sync.dma_start(rt[:], refk[b, :, :, sl])
                    ref_ts.append(rt)
                # one batch-wide candidate buffer + a single out-DMA per
                # tensor: the out-DMA then carries exactly one (RAW) wait,
                # and batch 1 gets a fresh slot (bufs=2) so there is no WAR
                # wait on top — DMA structs also allow only one sync wait
                vt = io.tile([128, NT, NCAND], mybir.dt.float32, tag="vals")
                it = io.tile([128, NT, NCAND], mybir.dt.uint16, tag="idx")
                for nt in range(NT):
                    for mt in range(MT):
                        # fresh slot for the first group of each batch: its
                        # matmul already needs the ref-DMA wait, and a PSUM
                        # WAR wait on top would exceed the 1-wait PE limit
                        # at the batch boundary (where the LDW slot is taken
                        # by the lr-DMA wait)
                        first = nt == 0 and mt == 0
                        ps = (psp1 if first else psp).tile(
                            [128, MTILE], mybir.dt.float32,
                            tag="ps1" if first else "ps")
                        for kc in range(KC):
                            nc.tensor.matmul(
                                ps,
                                lr_t[:, kc, nt * 128:(nt + 1) * 128],
                                ref_ts[mt][:, kc, :],
                                start=(kc == 0),
                                stop=(kc == KC - 1),
                            )
                        c8 = slice(mt * 8, (mt + 1) * 8)
                        nc.vector.max(out=vt[:, nt, c8], in_=ps)
                        nc.vector.max_index(
                            out=it[:, nt, c8], in_max=vt[:, nt, c8],
                            in_values=ps)
                nc.sync.dma_start(vals[b], vt)
                nc.sync.dma_start(idxs[b], it)
    if fix_waits:
        _cap_sync_waits(nc)
    return nc


LAST_EXEC_NS = None
LAST_TRACE = None


def kernel(lrsr_lv2, ref_lv2):
    import os
    global LAST_EXEC_NS, LAST_TRACE
    from concourse.bass_utils import run_bass_kernel_spmd

    lr_u = _unfold(np.asarray(lrsr_lv2, dtype=np.float32))
    ref_u = _unfold(np.asarray(ref_lv2, dtype=np.float32))

    ref64 = ref_u.astype(np.float64)
    lr64 = lr_u.astype(np.float64)
    ref_nrm = np.maximum(np.sqrt((ref64 * ref64).sum(1)), EPS)  # [B, HW]
    lr_nrm = np.maximum(np.sqrt((lr64 * lr64).sum(1)), EPS)     # [B, HW]
    refn64 = ref64 / ref_nrm[:, None, :]

    refk = _to_chunks_bf16(ref_u / ref_nrm[:, None, :].astype(np.float32))
    lrk = _to_chunks_bf16(lr_u)

    core_ids = list(range(NCORES))
    in_maps = [
        {"refk": refk,
         "lrk": np.ascontiguousarray(lrk[:, :, :, c * NPC:(c + 1) * NPC])}
        for c in core_ids
    ]
    nc = _build_bass()
    trace = os.environ.get("KERNEL_TRACE", "0") == "1"
    out = run_bass_kernel_spmd(nc, in_maps, core_ids, trace=trace)
    res = out.results
    LAST_EXEC_NS = out.exec_time_ns
    if out.instructions_and_trace is not None:
        LAST_TRACE = out.instructions_and_trace[1]

    # [B, HW] candidate values + local indices in global n order
    # (core-major, then n-tile, then partition); device layout is
    # [B, partition, n-tile, cand]
    vals = np.concatenate(
        [res[c]["vals"].transpose(0, 2, 1, 3).reshape(B, NPC, NCAND)
         for c in core_ids], axis=1)
    idxs = np.concatenate(
        [res[c]["idxs"].transpose(0, 2, 1, 3).reshape(B, NPC, NCAND)
         for c in core_ids], axis=1)

    # slot j of a candidate list came from m-tile j//8
    m_base = (np.arange(NCAND, dtype=np.int64) // 8) * MTILE
    bad = idxs >= MTILE  # unmatched MaxIndex slots (shouldn't happen)
    gidx = np.minimum(idxs.astype(np.int64), MTILE - 1) + m_base
    fvals = np.where(bad, -np.inf, vals)

    S = np.empty((B, HW), np.float32)
    Hm = np.empty((B, HW), np.int32)
    top = np.argpartition(-fvals, RESCORE - 1, axis=2)[:, :, :RESCORE]
    for b in range(B):
        cand = np.take_along_axis(gidx[b], top[b], axis=1)   # [HW, RESCORE]
        # exact rescore: scores[n, j] = <refn64[:, cand[n, j]], lr64[:, n]>
        CH = 1024
        for n0 in range(0, HW, CH):
            c = cand[n0:n0 + CH]                              # [CH, R]
            g = refn64[b][:, c]                               # [K, CH, R]
            sc = np.einsum("kcr,kc->cr", g, lr64[b][:, n0:n0 + CH])
            j = np.argmax(sc, axis=1)
            ar = np.arange(c.shape[0])
            S[b, n0:n0 + CH] = (sc[ar, j] / lr_nrm[b, n0:n0 + CH]).astype(
                np.float32)
            Hm[b, n0:n0 + CH] = c[ar, j].astype(np.int32)

    return (S.reshape(B, 1, H, W), Hm.reshape(B, 1, H, W))
